# revision 4
# baseline (speedup 1.0000x reference)
"""Trainium2 Bass kernel v3 for nn_MiddleLayerEncoder (gnn_message_passing).

Strategy: shard by CLUSTER across 8 cores (512 whole clusters each, no
collectives).  Host prep sorts points by cluster and packs whole clusters
into 1024-column bins (canonical across cores -> single SPMD program);
leftover bin columns are dead gaps that are never reduced.  Two bins make
a 2048-col supertile that shares a single L1 lhsT.

The L1 lhsT puts the per-supertile M rows FIRST (rows 0:NKM, addressed by
one-hot rows at the top of encT) and the static W1ab rows below, so the
M matmul result evacuates straight into the weight tile with a plain DVE
copy -- no partition-shift DMA on the critical path.  The L1 bias rides
in the M matmul via a ones row in neighT.

Segment maxes run as DVE reduce_max with 3D/4D access patterns directly
from PSUM per class-run of equal cluster width; enc2's two halves live in
one 2-bank PSUM tile so a single 4D reduce folds the A/B halves.  PSUM is
split into two role rings (enc+M / L1+L2) and the issue order keeps a
2-supertile software lookahead so per-engine FIFOs never head-of-line
block across stages.
"""

import numpy as np
import ml_dtypes
from contextlib import ExitStack

import concourse.bass as bass
import concourse.bacc as bacc
import concourse.tile as tile
from concourse import mybir
from concourse.bass_utils import run_bass_kernel_spmd

BF16 = mybir.dt.bfloat16
F32 = mybir.dt.float32
NPBF16 = ml_dtypes.bfloat16

N_CORES = 8
N_PTS = 262144
N_CLUSTERS = 4096
MINL = 36          # minimum padded cluster width (bounds clusters/supertile)
BIN = 1024         # cluster-aligned packing bin
ST = 2 * BIN       # supertile: shares one L1 lhsT
MAX_NK = 61        # one-hot rows limit: NKM + 67 <= 128

# L2 reduce routing: every ROUTE_PERIOD-th bin goes via a scalar-engine bf16
# copy + SBUF reduce instead of a direct DVE reduce from PSUM (Act/DVE load
# balance knob).
ROUTE_PERIOD = 10**9

# bf16 weight blob layout: (name, K rows, cols, partition base)
_WB16 = [
    ("enc1_lhsT", 12, 128, 0), ("enc2_lhsT", 64, 128, 0),
    ("enc2_hi", 64, 128, 64), ("W1ab", 67, 128, 0), ("W1cb1", 65, 128, 0),
    ("fcW2", 128, 128, 0), ("G1", 128, 128, 0),
    ("G2a", 128, 128, 0), ("G2b", 128, 128, 0),
]
_WF32 = ["b_enc1_4", "b_enc2", "b2", "gb1", "gb2a", "gb2b"]  # one f32 col each


# ---------------------------------------------------------------- planning

def _plan(cluster):
    """Canonical SPMD layout shared by all cores."""
    counts = np.bincount(cluster, minlength=N_CLUSTERS)
    assert counts.min() >= 1, "empty cluster unsupported"
    order = np.argsort(-counts, kind="stable")  # cluster ids, size desc

    n_ranks = N_CLUSTERS // N_CORES
    cids = np.empty((N_CORES, n_ranks), dtype=np.int64)
    for i, cid in enumerate(order):
        rnd, pos = divmod(i, N_CORES)
        core = pos if rnd % 2 == 0 else N_CORES - 1 - pos
        cids[core, rnd] = cid

    sizes = counts[cids]                      # [cores, ranks]
    L = sizes.max(axis=0)                     # canonical per-rank size
    L = np.maximum((L + 3) // 4 * 4, MINL).astype(np.int64)

    # pack ranks (size-desc order) into 1024-col bins of whole clusters;
    # leftover columns stay as a dead gap at the bin tail (never reduced)
    bins = []          # (r0, r1) rank ranges per bin
    col0 = np.zeros(n_ranks + 1, dtype=np.int64)
    r0 = 0
    while r0 < n_ranks:
        used = 0
        r1 = r0
        while r1 < n_ranks and used + L[r1] <= BIN:
            col0[r1] = BIN * len(bins) + used
            used += L[r1]
            r1 += 1
        assert r1 > r0
        bins.append((r0, r1))
        r0 = r1
    col0[n_ranks] = BIN * len(bins)
    S = BIN * len(bins)
    if len(bins) % 2:
        bins.append((n_ranks, n_ranks))  # empty bin: pad to whole supertiles
        S += BIN

    def runs(r0, r1):
        """Maximal runs of equal width: (rank_start, n, width)."""
        out = []
        i = r0
        while i < r1:
            j = i
            while j < r1 and L[j] == L[i]:
                j += 1
            out.append((i, j - i, int(L[i])))
            i = j
        return out

    bin_runs = [runs(b0, b1) for (b0, b1) in bins]
    # real (non-gap) columns per bin, rounded up to a multiple of 4
    bin_fill = [int(sum(L[b0:b1])) for (b0, b1) in bins]

    # supertiles: pairs of bins
    sts = []           # (r0, r1, c0)
    st_runs = []       # per supertile: runs broken at bin boundaries
    for b in range(0, len(bins), 2):
        r0, r1 = bins[b][0], bins[b + 1][1]
        sts.append((r0, r1, BIN * b))
        st_runs.append(bin_runs[b] + bin_runs[b + 1])
        assert r1 - r0 <= MAX_NK, f"supertile has {r1 - r0} clusters > {MAX_NK}"

    return dict(cids=cids, L=L, col0=col0, S=S, bins=bins, sts=sts,
                bin_runs=bin_runs, bin_fill=bin_fill, st_runs=st_runs,
                n_ranks=n_ranks, nkm=max(r1 - r0 for (r0, r1, _) in sts))


def _prep_core(k, plan, rel_points, features, sort_idx, bucket0):
    """Per-core input arrays (canonical layout, core-specific data)."""
    col0, S, L = plan["col0"], plan["S"], plan["L"]
    cids = plan["cids"][k]
    n_ranks = plan["n_ranks"]
    nkm = plan["nkm"]
    enc_rows = nkm + 67

    # gap columns (bin tails) keep index 0; they are never reduced
    slot = np.zeros(S, dtype=np.int64)
    for r in range(n_ranks):
        cid = cids[r]
        idx = sort_idx[bucket0[cid]: bucket0[cid + 1]]
        n = idx.shape[0]
        c0 = col0[r]
        slot[c0: c0 + n] = idx
        if L[r] > n:
            slot[c0 + n: c0 + L[r]] = idx[0]

    pts = rel_points[slot]          # [S, 3] f32
    feat = features[slot]           # [S, 64] f32

    # encT rows: 0:nkm one-hot (local rank in supertile), nkm:nkm+3 points,
    # nkm+3:nkm+67 features
    encT = np.zeros((enc_rows, S), dtype=NPBF16)
    encT[nkm:nkm + 3] = pts.T.astype(NPBF16)
    encT[nkm + 3:nkm + 67] = feat.T.astype(NPBF16)
    oh_row = np.full(S, nkm - 1, dtype=np.int64)  # gap cols: harmless row
    for (r0, r1, c0) in plan["sts"]:
        for r in range(r0, r1):
            oh_row[col0[r]:col0[r] + L[r]] = r - r0
    encT[oh_row, np.arange(S)] = NPBF16(1.0)

    pts4 = (
        pts.astype(NPBF16)
        .reshape(S // 4, 4, 3)
        .transpose(1, 2, 0)
        .reshape(12, S // 4)
    )
    return {"encT": encT, "pts4": np.ascontiguousarray(pts4)}


def _blockdiag(w, times):
    fi, fo = w.shape
    out = np.zeros((fi * times, fo * times), dtype=w.dtype)
    for i in range(times):
        out[i * fi:(i + 1) * fi, i * fo:(i + 1) * fo] = w
    return out


def _prep_weights(inp):
    W1 = inp["W1"]
    mats = {
        "enc1_lhsT": _blockdiag(inp["enc_W1"], 4),
        "enc2_lhsT": _blockdiag(inp["enc_W2"], 2),
        "enc2_hi": _blockdiag(inp["enc_W2"], 2),
        "W1ab": W1[0:67],
        "W1cb1": np.vstack([W1[67:131], inp["b1"][None]]),
        "fcW2": inp["W2"], "G1": inp["G1"],
        "G2a": inp["G2"][:, 0:128], "G2b": inp["G2"][:, 128:256],
    }
    wb16 = np.zeros((128, 128 * len(_WB16)), dtype=NPBF16)
    for i, (name, k, cols, pbase) in enumerate(_WB16):
        wb16[pbase:pbase + k, 128 * i:128 * i + cols] = mats[name].astype(NPBF16)

    vecs = {
        "b_enc1_4": np.tile(inp["enc_b1"], 4), "b_enc2": inp["enc_b2"],
        "b2": inp["b2"], "gb1": inp["gb1"],
        "gb2a": inp["gb2"][0:128], "gb2b": inp["gb2"][128:256],
    }
    wf32 = np.zeros((128, len(_WF32)), dtype=np.float32)
    for i, name in enumerate(_WF32):
        v = vecs[name]
        wf32[0:v.shape[0], i] = v
    return {"wb16": wb16, "wf32": wf32}


# ---------------------------------------------------------------- program

def _build(plan):
    S = plan["S"]
    n_ranks = plan["n_ranks"]
    nkm = plan["nkm"]
    enc_rows = nkm + 67
    nc = bacc.Bacc(None, target_bir_lowering=False, debug=True)

    encT_d = nc.dram_tensor("encT", [enc_rows, S], BF16, kind="ExternalInput")
    pts4_d = nc.dram_tensor("pts4", [12, S // 4], BF16, kind="ExternalInput")
    wb16_d = nc.dram_tensor("wb16", [128, 128 * len(_WB16)], BF16,
                            kind="ExternalInput")
    wf32_d = nc.dram_tensor("wf32", [128, len(_WF32)], F32, kind="ExternalInput")
    out_d = nc.dram_tensor("out", [256, 512], F32, kind="ExternalOutput")

    RELU = mybir.ActivationFunctionType.Relu
    COPY = mybir.ActivationFunctionType.Copy
    ADD = mybir.AluOpType.add
    MAX = mybir.AluOpType.max
    AX = mybir.AxisListType.X
    AXY = mybir.AxisListType.XY

    sts = plan["sts"]
    n_st = len(sts)
    col0 = plan["col0"]
    W1AB_COL = 128 * 3  # W1ab offset in the bf16 blob

    with tile.TileContext(nc) as tc, ExitStack() as ctx:
        consts = ctx.enter_context(tc.tile_pool(name="consts", bufs=1))
        glob = ctx.enter_context(tc.tile_pool(name="glob", bufs=1))
        enc_p = ctx.enter_context(tc.tile_pool(name="enc_p", bufs=3))
        h1_p = ctx.enter_context(tc.tile_pool(name="h1_p", bufs=2))
        e1_p = ctx.enter_context(tc.tile_pool(name="e1_p", bufs=3))
        sm_p = ctx.enter_context(tc.tile_pool(name="sm_p", bufs=3))
        cp_p = ctx.enter_context(tc.tile_pool(name="cp_p", bufs=2))
        psE = ctx.enter_context(tc.tile_pool(name="psE", bufs=1, space="PSUM"))
        psL = ctx.enter_context(tc.tile_pool(name="psL", bufs=3, space="PSUM"))

        wb16_t = consts.tile([128, 128 * len(_WB16)], BF16, tag="wb16")
        nc.sync.dma_start(out=wb16_t[:], in_=wb16_d[:])
        wf32_t = consts.tile([128, len(_WF32)], F32, tag="wf32")
        nc.sync.dma_start(out=wf32_t[:], in_=wf32_d[:])

        w_sb = {}
        for i, (name, k, cols, pbase) in enumerate(_WB16):
            w_sb[name] = wb16_t[pbase:pbase + k, 128 * i:128 * i + cols]
        bias = {}
        for i, name in enumerate(_WF32):
            rows = {"b_enc2": 64}.get(name, 128)
            bias[name] = wf32_t[0:rows, i:i + 1]

        # resident quad-packed points
        pts4_sb = consts.tile([12, S // 4], BF16, tag="pts4_sb")
        nc.sync.dma_start(out=pts4_sb[:], in_=pts4_d[:])

        # L1 lhsT slots: rows 0:nkm = per-st M (DVE-copied in), rows
        # nkm:nkm+67 = W1ab (static preload via the scalar engine's HWDGE)
        lhsT_slots = [consts.tile([128, 128], BF16, tag=f"lhsT{i}",
                                  name=f"lhsT{i}") for i in range(3)]
        for sl in lhsT_slots:
            # zero the M rows: rows nk:nkm are multiplied by zero encT rows,
            # but uninitialized SBUF could hold Inf/NaN (0*Inf = NaN)
            nc.vector.memset(sl[0:nkm, :], 0.0)
            nc.scalar.dma_start(out=sl[nkm:nkm + 67, :],
                                in_=wb16_d[0:67, W1AB_COL:W1AB_COL + 128])

        # neighT: [65, n_ranks]; row 64 = ones (folds b1 into the M matmul)
        neighT = glob.tile([65, n_ranks], BF16, tag="neighT")
        nc.vector.memset(neighT[64:65, :], 1.0)
        # stage-2 partials for the whole core
        T2 = glob.tile([128, n_ranks], BF16, tag="T2")

        # --------------- pipeline stages (per supertile) ---------------

        def enc_head(si):
            """enc1 matmul + relu evac; returns the shared enc psum tile."""
            (r0, r1, c0) = sts[si]
            q0 = c0 // 4   # 512 quads per supertile
            ab = psE.tile([128, BIN], F32, tag="psE", name="ab")
            nc.tensor.matmul(ab[:, 0:512], w_sb["enc1_lhsT"],
                             pts4_sb[:, q0:q0 + 512], start=True, stop=True)
            h1 = h1_p.tile([128, 512], BF16, tag="h1")
            nc.scalar.activation(h1[:], ab[:, 0:512], RELU,
                                 bias=bias["b_enc1_4"], scale=1.0)
            # prefetch this supertile's encT
            encT_t = enc_p.tile([enc_rows, ST], BF16, tag="encT_t")
            nc.sync.dma_start(out=encT_t[:], in_=encT_d[:, c0:c0 + ST])
            return ab, h1, encT_t

        def enc_tail(si, ab, h1):
            """enc2 matmuls into the shared psum tile + stage-1 reduce ->
            neighT columns for this supertile."""
            (r0, r1, c0) = sts[si]
            nk = r1 - r0
            nc.tensor.matmul(ab[:, 0:512], w_sb["enc2_lhsT"], h1[0:64, :],
                             start=True, stop=True)
            nc.tensor.matmul(ab[:, 512:1024], w_sb["enc2_hi"], h1[64:128, :],
                             start=True, stop=True)
            # fused A/B reduce: [p, n, 2(half), wq] -> max over (half, wq)
            mx = sm_p.tile([128, MAX_NK], BF16, tag="mx")
            halves = ab[:].rearrange("p (h c) -> p h c", h=2)
            for (rr, n, w) in plan["st_runs"][si]:
                wq = w // 4
                o = (int(col0[rr]) - c0) // 4
                nc.vector.reduce_max(
                    mx[:, rr - r0: rr - r0 + n],
                    halves[:, :, o: o + n * wq]
                    .rearrange("p h (n w) -> p n h w", w=wq),
                    axis=AXY)
            fold = sm_p.tile([64, MAX_NK], BF16, tag="fold")
            nc.sync.dma_start(out=fold[:, :nk], in_=mx[64:128, :nk])
            mx2 = sm_p.tile([64, MAX_NK], BF16, tag="mx2")
            nc.vector.tensor_max(mx2[:, :nk], mx[0:64, :nk], fold[:, :nk])
            nc.vector.tensor_scalar(neighT[0:64, r0:r1], mx2[:, :nk],
                                    bias["b_enc2"], 0.0, op0=ADD, op1=MAX)

        def m_chain(si):
            """M = [neighT;1].T @ [W1c;b1], evac'd into the lhsT slot rows."""
            (r0, r1, c0) = sts[si]
            nk = r1 - r0
            pm = psL.tile([128, BIN], F32, tag="psL", name="pm")
            nc.tensor.matmul(pm[:nk, 0:128], neighT[0:65, r0:r1],
                             w_sb["W1cb1"], start=True, stop=True)
            nc.vector.tensor_copy(lhsT_slots[si % 3][0:nk, :], pm[:nk, 0:128])

        def l1_part(si, encT_t):
            """L1 matmuls for both bins first, relu evacs trail behind.
            Dead gap columns at each bin tail are skipped entirely."""
            (r0, r1, c0) = sts[si]
            slot = lhsT_slots[si % 3]
            e1 = e1_p.tile([128, ST], BF16, tag="e1")
            p1s = []
            for t in (0, 1):
                fill = plan["bin_fill"][c0 // BIN + t]
                p1 = psL.tile([128, BIN], F32, tag="psL", name="p1")
                for a in (0, 512):
                    z = min(512, fill - a)
                    if z > 0:
                        nc.tensor.matmul(
                            p1[:, a:a + z], slot[0:enc_rows, :],
                            encT_t[:, t * BIN + a:t * BIN + a + z],
                            start=True, stop=True)
                p1s.append((p1, fill))
            for t in (0, 1):
                p1, fill = p1s[t]
                if fill > 0:
                    nc.scalar.activation(e1[:, t * BIN:t * BIN + fill],
                                         p1[:, :fill], RELU)
            return e1

        def l2_part(si, e1):
            (r0, r1, c0) = sts[si]
            for t in (0, 1):
                bin_idx = c0 // BIN + t
                fill = plan["bin_fill"][bin_idx]
                p2 = psL.tile([128, BIN], F32, tag="psL", name="p2")
                for a in (0, 512):
                    z = min(512, fill - a)
                    if z > 0:
                        nc.tensor.matmul(
                            p2[:, a:a + z], w_sb["fcW2"],
                            e1[:, t * BIN + a:t * BIN + a + z],
                            start=True, stop=True)
                bc0 = BIN * bin_idx
                if fill == 0:
                    continue
                if bin_idx % ROUTE_PERIOD == ROUTE_PERIOD - 1:
                    cp = cp_p.tile([128, BIN], BF16, tag="cp")
                    nc.scalar.activation(cp[:, :fill], p2[:, :fill], COPY)
                    src = cp
                else:
                    src = p2
                for (rr, n, w) in plan["bin_runs"][bin_idx]:
                    o = int(col0[rr]) - bc0
                    nc.vector.reduce_max(
                        T2[:, rr:rr + n],
                        src[:, o: o + n * w].rearrange("p (n w) -> p n w", w=w),
                        axis=AX)

        # --------------- software pipeline, lookahead 2 ---------------
        pend = {}
        for si in range(min(2, n_st)):
            pend[si] = enc_head(si)
            enc_tail(si, pend[si][0], pend[si][1])
        if n_st > 0:
            m_chain(0)
        for si in range(n_st):
            if si + 2 < n_st:
                pend[si + 2] = enc_head(si + 2)
            e1 = l1_part(si, pend[si][2])
            if si + 1 < n_st:
                m_chain(si + 1)
            l2_part(si, e1)
            if si + 2 < n_st:
                enc_tail(si + 2, pend[si + 2][0], pend[si + 2][1])
            pend.pop(si)

        # ---------------- global MLP tail ----------------
        gT = glob.tile([128, n_ranks], BF16, tag="gT")
        nc.vector.tensor_scalar(gT[:], T2[:], bias["b2"], 0.0, op0=ADD, op1=MAX)
        pg = psL.tile([128, BIN], F32, tag="psL", name="pg")
        nc.tensor.matmul(pg[:, 0:512], w_sb["G1"], gT[:], start=True, stop=True)
        g1T = glob.tile([128, 512], BF16, tag="g1T")
        nc.scalar.activation(g1T[:], pg[:, 0:512], RELU, bias=bias["gb1"],
                             scale=1.0)
        for half, (wn, bn) in enumerate((("G2a", "gb2a"), ("G2b", "gb2b"))):
            po = psL.tile([128, BIN], F32, tag="psL", name="po")
            nc.tensor.matmul(po[:, 0:512], w_sb[wn], g1T[:], start=True,
                             stop=True)
            o_sb = glob.tile([128, 512], F32, tag=f"osb{half}")
            nc.scalar.activation(o_sb[:], po[:, 0:512], RELU, bias=bias[bn],
                                 scale=1.0)
            nc.sync.dma_start(out=out_d[half * 128:(half + 1) * 128, :],
                              in_=o_sb[:])

    nc.finalize()
    return nc


# ---------------------------------------------------------------- entry

_CACHE = {}


def _run(inputs, trace=False, **spmd_kwargs):
    cluster = np.asarray(inputs["cluster"])
    key = hash(cluster.tobytes())
    if key not in _CACHE:
        plan = _plan(cluster)
        nc = _build(plan)
        _CACHE[key] = (plan, nc)
    plan, nc = _CACHE[key]

    rel_points = np.asarray(inputs["relative_points"], dtype=np.float32)
    features = np.asarray(inputs["features"], dtype=np.float32)
    sort_idx = np.argsort(cluster, kind="stable")
    bucket0 = np.concatenate(
        [[0], np.cumsum(np.bincount(cluster, minlength=N_CLUSTERS))]
    )
    wmap = _prep_weights({k: np.asarray(v, dtype=np.float32)
                          for k, v in inputs.items()
                          if k not in ("relative_points", "features", "cluster")})

    in_maps = []
    for k in range(N_CORES):
        m = _prep_core(k, plan, rel_points, features, sort_idx, bucket0)
        m.update(wmap)
        in_maps.append(m)

    res = run_bass_kernel_spmd(nc, in_maps, list(range(N_CORES)),
                               trace=trace, **spmd_kwargs)

    out = np.empty((N_CLUSTERS, 256), dtype=np.float32)
    for k in range(N_CORES):
        out[plan["cids"][k]] = res.results[k]["out"].T
    return out, res


def kernel(**inputs):
    return _run(inputs)[0]


# revision 5
# speedup vs baseline: 1.0946x; 1.0946x over previous
"""Trainium2 Bass kernel v3 for nn_MiddleLayerEncoder (gnn_message_passing).

Strategy: shard by CLUSTER across 8 cores (512 whole clusters each, no
collectives).  Host prep sorts points by cluster and packs whole clusters
into 1024-column bins (canonical across cores -> single SPMD program);
leftover bin columns are dead gaps that are never reduced.  Two bins make
a 2048-col supertile that shares a single L1 lhsT.

The L1 lhsT puts the per-supertile M rows FIRST (rows 0:NKM, addressed by
one-hot rows at the top of encT) and the static W1ab rows below, so the
M matmul result evacuates straight into the weight tile with a plain DVE
copy -- no partition-shift DMA on the critical path.  The L1 bias rides
in the M matmul via a ones row in neighT.

Segment maxes run as DVE reduce_max with 3D/4D access patterns directly
from PSUM per class-run of equal cluster width; enc2's two halves live in
one 2-bank PSUM tile so a single 4D reduce folds the A/B halves.  PSUM is
split into two role rings (enc+M / L1+L2) and the issue order keeps a
2-supertile software lookahead so per-engine FIFOs never head-of-line
block across stages.
"""

import numpy as np
import ml_dtypes
from contextlib import ExitStack

import concourse.bass as bass
import concourse.bacc as bacc
import concourse.tile as tile
from concourse import mybir
from concourse.bass_utils import run_bass_kernel_spmd

BF16 = mybir.dt.bfloat16
F32 = mybir.dt.float32
NPBF16 = ml_dtypes.bfloat16

N_CORES = 8
N_PTS = 262144
N_CLUSTERS = 4096
MINL = 36          # minimum padded cluster width (bounds clusters/supertile)
BIN = 1024         # cluster-aligned packing bin
ST = 2 * BIN       # supertile: shares one L1 lhsT
MAX_NK = 61        # one-hot rows limit: NKM + 67 <= 128

# L2 reduce routing: every ROUTE_PERIOD-th bin goes via a scalar-engine bf16
# copy + SBUF reduce instead of a direct DVE reduce from PSUM (Act/DVE load
# balance knob).
ROUTE_PERIOD = 2

# bf16 weight blob layout: (name, K rows, cols, partition base)
_WB16 = [
    ("enc1_lhsT", 12, 128, 0), ("enc2_lhsT", 64, 128, 0),
    ("enc2_hi", 64, 128, 64), ("W1ab", 67, 128, 0), ("W1cb1", 65, 128, 0),
    ("fcW2", 128, 128, 0), ("G1", 128, 128, 0),
    ("G2a", 128, 128, 0), ("G2b", 128, 128, 0),
]
_WF32 = ["b_enc1_4", "b_enc2", "b2", "gb1", "gb2a", "gb2b"]  # one f32 col each


# ---------------------------------------------------------------- planning

def _plan(cluster):
    """Canonical SPMD layout shared by all cores."""
    counts = np.bincount(cluster, minlength=N_CLUSTERS)
    assert counts.min() >= 1, "empty cluster unsupported"
    order = np.argsort(-counts, kind="stable")  # cluster ids, size desc

    n_ranks = N_CLUSTERS // N_CORES
    cids = np.empty((N_CORES, n_ranks), dtype=np.int64)
    for i, cid in enumerate(order):
        rnd, pos = divmod(i, N_CORES)
        core = pos if rnd % 2 == 0 else N_CORES - 1 - pos
        cids[core, rnd] = cid

    sizes = counts[cids]                      # [cores, ranks]
    L = sizes.max(axis=0)                     # canonical per-rank size
    L = np.maximum((L + 3) // 4 * 4, MINL).astype(np.int64)

    # pack ranks (size-desc order) into 1024-col bins of whole clusters;
    # leftover columns stay as a dead gap at the bin tail (never reduced)
    bins = []          # (r0, r1) rank ranges per bin
    col0 = np.zeros(n_ranks + 1, dtype=np.int64)
    r0 = 0
    while r0 < n_ranks:
        used = 0
        r1 = r0
        while r1 < n_ranks and used + L[r1] <= BIN:
            col0[r1] = BIN * len(bins) + used
            used += L[r1]
            r1 += 1
        assert r1 > r0
        bins.append((r0, r1))
        r0 = r1
    col0[n_ranks] = BIN * len(bins)
    S = BIN * len(bins)
    if len(bins) % 2:
        bins.append((n_ranks, n_ranks))  # empty bin: pad to whole supertiles
        S += BIN

    def runs(r0, r1):
        """Maximal runs of equal width: (rank_start, n, width)."""
        out = []
        i = r0
        while i < r1:
            j = i
            while j < r1 and L[j] == L[i]:
                j += 1
            out.append((i, j - i, int(L[i])))
            i = j
        return out

    bin_runs = [runs(b0, b1) for (b0, b1) in bins]
    # real (non-gap) columns per bin, rounded up to a multiple of 4
    bin_fill = [int(sum(L[b0:b1])) for (b0, b1) in bins]

    # supertiles: pairs of bins
    sts = []           # (r0, r1, c0)
    st_runs = []       # per supertile: runs broken at bin boundaries
    for b in range(0, len(bins), 2):
        r0, r1 = bins[b][0], bins[b + 1][1]
        sts.append((r0, r1, BIN * b))
        st_runs.append(bin_runs[b] + bin_runs[b + 1])
        assert r1 - r0 <= MAX_NK, f"supertile has {r1 - r0} clusters > {MAX_NK}"

    return dict(cids=cids, L=L, col0=col0, S=S, bins=bins, sts=sts,
                bin_runs=bin_runs, bin_fill=bin_fill, st_runs=st_runs,
                n_ranks=n_ranks, nkm=max(r1 - r0 for (r0, r1, _) in sts))


def _prep_core(k, plan, rel_points, features, sort_idx, bucket0):
    """Per-core input arrays (canonical layout, core-specific data)."""
    col0, S, L = plan["col0"], plan["S"], plan["L"]
    cids = plan["cids"][k]
    n_ranks = plan["n_ranks"]
    nkm = plan["nkm"]
    enc_rows = nkm + 67

    # gap columns (bin tails) keep index 0; they are never reduced
    slot = np.zeros(S, dtype=np.int64)
    for r in range(n_ranks):
        cid = cids[r]
        idx = sort_idx[bucket0[cid]: bucket0[cid + 1]]
        n = idx.shape[0]
        c0 = col0[r]
        slot[c0: c0 + n] = idx
        if L[r] > n:
            slot[c0 + n: c0 + L[r]] = idx[0]

    pts = rel_points[slot]          # [S, 3] f32
    feat = features[slot]           # [S, 64] f32

    # encT rows: 0:nkm one-hot (local rank in supertile), nkm:nkm+3 points,
    # nkm+3:nkm+67 features
    encT = np.zeros((enc_rows, S), dtype=NPBF16)
    encT[nkm:nkm + 3] = pts.T.astype(NPBF16)
    encT[nkm + 3:nkm + 67] = feat.T.astype(NPBF16)
    oh_row = np.full(S, nkm - 1, dtype=np.int64)  # gap cols: harmless row
    for (r0, r1, c0) in plan["sts"]:
        for r in range(r0, r1):
            oh_row[col0[r]:col0[r] + L[r]] = r - r0
    encT[oh_row, np.arange(S)] = NPBF16(1.0)

    pts4 = (
        pts.astype(NPBF16)
        .reshape(S // 4, 4, 3)
        .transpose(1, 2, 0)
        .reshape(12, S // 4)
    )
    return {"encT": encT, "pts4": np.ascontiguousarray(pts4)}


def _blockdiag(w, times):
    fi, fo = w.shape
    out = np.zeros((fi * times, fo * times), dtype=w.dtype)
    for i in range(times):
        out[i * fi:(i + 1) * fi, i * fo:(i + 1) * fo] = w
    return out


def _prep_weights(inp):
    W1 = inp["W1"]
    mats = {
        "enc1_lhsT": _blockdiag(inp["enc_W1"], 4),
        "enc2_lhsT": _blockdiag(inp["enc_W2"], 2),
        "enc2_hi": _blockdiag(inp["enc_W2"], 2),
        "W1ab": W1[0:67],
        "W1cb1": np.vstack([W1[67:131], inp["b1"][None]]),
        "fcW2": inp["W2"], "G1": inp["G1"],
        "G2a": inp["G2"][:, 0:128], "G2b": inp["G2"][:, 128:256],
    }
    wb16 = np.zeros((128, 128 * len(_WB16)), dtype=NPBF16)
    for i, (name, k, cols, pbase) in enumerate(_WB16):
        wb16[pbase:pbase + k, 128 * i:128 * i + cols] = mats[name].astype(NPBF16)

    vecs = {
        "b_enc1_4": np.tile(inp["enc_b1"], 4), "b_enc2": inp["enc_b2"],
        "b2": inp["b2"], "gb1": inp["gb1"],
        "gb2a": inp["gb2"][0:128], "gb2b": inp["gb2"][128:256],
    }
    wf32 = np.zeros((128, len(_WF32)), dtype=np.float32)
    for i, name in enumerate(_WF32):
        v = vecs[name]
        wf32[0:v.shape[0], i] = v
    return {"wb16": wb16, "wf32": wf32}


# ---------------------------------------------------------------- program

def _build(plan):
    S = plan["S"]
    n_ranks = plan["n_ranks"]
    nkm = plan["nkm"]
    enc_rows = nkm + 67
    nc = bacc.Bacc(None, target_bir_lowering=False, debug=True)

    encT_d = nc.dram_tensor("encT", [enc_rows, S], BF16, kind="ExternalInput")
    pts4_d = nc.dram_tensor("pts4", [12, S // 4], BF16, kind="ExternalInput")
    wb16_d = nc.dram_tensor("wb16", [128, 128 * len(_WB16)], BF16,
                            kind="ExternalInput")
    wf32_d = nc.dram_tensor("wf32", [128, len(_WF32)], F32, kind="ExternalInput")
    out_d = nc.dram_tensor("out", [256, 512], F32, kind="ExternalOutput")

    RELU = mybir.ActivationFunctionType.Relu
    COPY = mybir.ActivationFunctionType.Copy
    ADD = mybir.AluOpType.add
    MAX = mybir.AluOpType.max
    AX = mybir.AxisListType.X
    AXY = mybir.AxisListType.XY

    sts = plan["sts"]
    n_st = len(sts)
    col0 = plan["col0"]
    W1AB_COL = 128 * 3  # W1ab offset in the bf16 blob

    with tile.TileContext(nc) as tc, ExitStack() as ctx:
        consts = ctx.enter_context(tc.tile_pool(name="consts", bufs=1))
        glob = ctx.enter_context(tc.tile_pool(name="glob", bufs=1))
        enc_p = ctx.enter_context(tc.tile_pool(name="enc_p", bufs=3))
        h1_p = ctx.enter_context(tc.tile_pool(name="h1_p", bufs=2))
        e1_p = ctx.enter_context(tc.tile_pool(name="e1_p", bufs=2))
        sm_p = ctx.enter_context(tc.tile_pool(name="sm_p", bufs=3))
        cp_p = ctx.enter_context(tc.tile_pool(name="cp_p", bufs=2))
        psE = ctx.enter_context(tc.tile_pool(name="psE", bufs=1, space="PSUM"))
        psL = ctx.enter_context(tc.tile_pool(name="psL", bufs=3, space="PSUM"))

        wb16_t = consts.tile([128, 128 * len(_WB16)], BF16, tag="wb16")
        nc.sync.dma_start(out=wb16_t[:], in_=wb16_d[:])
        wf32_t = consts.tile([128, len(_WF32)], F32, tag="wf32")
        nc.sync.dma_start(out=wf32_t[:], in_=wf32_d[:])

        w_sb = {}
        for i, (name, k, cols, pbase) in enumerate(_WB16):
            w_sb[name] = wb16_t[pbase:pbase + k, 128 * i:128 * i + cols]
        bias = {}
        for i, name in enumerate(_WF32):
            rows = {"b_enc2": 64}.get(name, 128)
            bias[name] = wf32_t[0:rows, i:i + 1]

        # resident quad-packed points
        pts4_sb = consts.tile([12, S // 4], BF16, tag="pts4_sb")
        nc.sync.dma_start(out=pts4_sb[:], in_=pts4_d[:])

        # L1 lhsT slots: rows 0:nkm = per-st M (DVE-copied in), rows
        # nkm:nkm+67 = W1ab (static preload via the scalar engine's HWDGE)
        lhsT_slots = [consts.tile([128, 128], BF16, tag=f"lhsT{i}",
                                  name=f"lhsT{i}") for i in range(3)]
        for sl in lhsT_slots:
            # zero the M rows: rows nk:nkm are multiplied by zero encT rows,
            # but uninitialized SBUF could hold Inf/NaN (0*Inf = NaN)
            nc.vector.memset(sl[0:nkm, :], 0.0)
            nc.scalar.dma_start(out=sl[nkm:nkm + 67, :],
                                in_=wb16_d[0:67, W1AB_COL:W1AB_COL + 128])

        # neighT: [65, n_ranks]; row 64 = ones (folds b1 into the M matmul)
        neighT = glob.tile([65, n_ranks], BF16, tag="neighT")
        nc.vector.memset(neighT[64:65, :], 1.0)
        # stage-2 partials for the whole core
        T2 = glob.tile([128, n_ranks], BF16, tag="T2")

        # --------------- pipeline stages (per supertile) ---------------

        def enc_head(si):
            """enc1 matmul + relu evac; returns the shared enc psum tile."""
            (r0, r1, c0) = sts[si]
            q0 = c0 // 4   # 512 quads per supertile
            ab = psE.tile([128, BIN], F32, tag="psE", name="ab")
            nc.tensor.matmul(ab[:, 0:512], w_sb["enc1_lhsT"],
                             pts4_sb[:, q0:q0 + 512], start=True, stop=True)
            h1 = h1_p.tile([128, 512], BF16, tag="h1")
            nc.scalar.activation(h1[:], ab[:, 0:512], RELU,
                                 bias=bias["b_enc1_4"], scale=1.0)
            # prefetch this supertile's encT
            encT_t = enc_p.tile([enc_rows, ST], BF16, tag="encT_t")
            nc.sync.dma_start(out=encT_t[:], in_=encT_d[:, c0:c0 + ST])
            return ab, h1, encT_t

        def enc_tail(si, ab, h1):
            """enc2 matmuls into the shared psum tile + stage-1 reduce ->
            neighT columns for this supertile."""
            (r0, r1, c0) = sts[si]
            nk = r1 - r0
            nc.tensor.matmul(ab[:, 0:512], w_sb["enc2_lhsT"], h1[0:64, :],
                             start=True, stop=True)
            nc.tensor.matmul(ab[:, 512:1024], w_sb["enc2_hi"], h1[64:128, :],
                             start=True, stop=True)
            # fused A/B reduce: [p, n, 2(half), wq] -> max over (half, wq)
            mx = sm_p.tile([128, MAX_NK], BF16, tag="mx")
            halves = ab[:].rearrange("p (h c) -> p h c", h=2)
            for (rr, n, w) in plan["st_runs"][si]:
                wq = w // 4
                o = (int(col0[rr]) - c0) // 4
                nc.vector.reduce_max(
                    mx[:, rr - r0: rr - r0 + n],
                    halves[:, :, o: o + n * wq]
                    .rearrange("p h (n w) -> p n h w", w=wq),
                    axis=AXY)
            fold = sm_p.tile([64, MAX_NK], BF16, tag="fold")
            nc.sync.dma_start(out=fold[:, :nk], in_=mx[64:128, :nk])
            mx2 = sm_p.tile([64, MAX_NK], BF16, tag="mx2")
            nc.vector.tensor_max(mx2[:, :nk], mx[0:64, :nk], fold[:, :nk])
            nc.vector.tensor_scalar(neighT[0:64, r0:r1], mx2[:, :nk],
                                    bias["b_enc2"], 0.0, op0=ADD, op1=MAX)

        def m_chain(si):
            """M = [neighT;1].T @ [W1c;b1], evac'd into the lhsT slot rows."""
            (r0, r1, c0) = sts[si]
            nk = r1 - r0
            pm = psL.tile([128, BIN], F32, tag="psL", name="pm")
            nc.tensor.matmul(pm[:nk, 0:128], neighT[0:65, r0:r1],
                             w_sb["W1cb1"], start=True, stop=True)
            nc.vector.tensor_copy(lhsT_slots[si % 3][0:nk, :], pm[:nk, 0:128])

        def l1_part(si, encT_t):
            """L1 matmuls for both bins first, relu evacs trail behind.
            Dead gap columns at each bin tail are skipped entirely."""
            (r0, r1, c0) = sts[si]
            slot = lhsT_slots[si % 3]
            e1 = e1_p.tile([128, ST], BF16, tag="e1")
            p1s = []
            for t in (0, 1):
                fill = plan["bin_fill"][c0 // BIN + t]
                p1 = psL.tile([128, BIN], F32, tag="psL", name="p1")
                for a in (0, 512):
                    z = min(512, fill - a)
                    if z > 0:
                        nc.tensor.matmul(
                            p1[:, a:a + z], slot[0:enc_rows, :],
                            encT_t[:, t * BIN + a:t * BIN + a + z],
                            start=True, stop=True)
                p1s.append((p1, fill))
            for t in (0, 1):
                p1, fill = p1s[t]
                if fill > 0:
                    nc.scalar.activation(e1[:, t * BIN:t * BIN + fill],
                                         p1[:, :fill], RELU)
            return e1

        def l2_part(si, e1):
            (r0, r1, c0) = sts[si]
            for t in (0, 1):
                bin_idx = c0 // BIN + t
                fill = plan["bin_fill"][bin_idx]
                p2 = psL.tile([128, BIN], F32, tag="psL", name="p2")
                for a in (0, 512):
                    z = min(512, fill - a)
                    if z > 0:
                        nc.tensor.matmul(
                            p2[:, a:a + z], w_sb["fcW2"],
                            e1[:, t * BIN + a:t * BIN + a + z],
                            start=True, stop=True)
                bc0 = BIN * bin_idx
                if fill == 0:
                    continue
                if bin_idx % ROUTE_PERIOD == ROUTE_PERIOD - 1:
                    cp = cp_p.tile([128, BIN], BF16, tag="cp")
                    nc.scalar.activation(cp[:, :fill], p2[:, :fill], COPY)
                    src = cp
                else:
                    src = p2
                for (rr, n, w) in plan["bin_runs"][bin_idx]:
                    o = int(col0[rr]) - bc0
                    nc.vector.reduce_max(
                        T2[:, rr:rr + n],
                        src[:, o: o + n * w].rearrange("p (n w) -> p n w", w=w),
                        axis=AX)

        # --------------- software pipeline, lookahead 2 ---------------
        pend = {}
        for si in range(min(2, n_st)):
            pend[si] = enc_head(si)
            enc_tail(si, pend[si][0], pend[si][1])
        if n_st > 0:
            m_chain(0)
        for si in range(n_st):
            if si + 2 < n_st:
                pend[si + 2] = enc_head(si + 2)
            e1 = l1_part(si, pend[si][2])
            l2_part(si, e1)
            if si + 1 < n_st:
                m_chain(si + 1)
            if si + 2 < n_st:
                enc_tail(si + 2, pend[si + 2][0], pend[si + 2][1])
            pend.pop(si)

        # ---------------- global MLP tail ----------------
        gT = glob.tile([128, n_ranks], BF16, tag="gT")
        nc.vector.tensor_scalar(gT[:], T2[:], bias["b2"], 0.0, op0=ADD, op1=MAX)
        pg = psL.tile([128, BIN], F32, tag="psL", name="pg")
        nc.tensor.matmul(pg[:, 0:512], w_sb["G1"], gT[:], start=True, stop=True)
        g1T = glob.tile([128, 512], BF16, tag="g1T")
        nc.scalar.activation(g1T[:], pg[:, 0:512], RELU, bias=bias["gb1"],
                             scale=1.0)
        for half, (wn, bn) in enumerate((("G2a", "gb2a"), ("G2b", "gb2b"))):
            po = psL.tile([128, BIN], F32, tag="psL", name="po")
            nc.tensor.matmul(po[:, 0:512], w_sb[wn], g1T[:], start=True,
                             stop=True)
            o_sb = glob.tile([128, 512], F32, tag=f"osb{half}")
            nc.scalar.activation(o_sb[:], po[:, 0:512], RELU, bias=bias[bn],
                                 scale=1.0)
            nc.sync.dma_start(out=out_d[half * 128:(half + 1) * 128, :],
                              in_=o_sb[:])

    nc.finalize()
    return nc


# ---------------------------------------------------------------- entry

_CACHE = {}


def _run(inputs, trace=False, **spmd_kwargs):
    cluster = np.asarray(inputs["cluster"])
    key = hash(cluster.tobytes())
    if key not in _CACHE:
        plan = _plan(cluster)
        nc = _build(plan)
        _CACHE[key] = (plan, nc)
    plan, nc = _CACHE[key]

    rel_points = np.asarray(inputs["relative_points"], dtype=np.float32)
    features = np.asarray(inputs["features"], dtype=np.float32)
    sort_idx = np.argsort(cluster, kind="stable")
    bucket0 = np.concatenate(
        [[0], np.cumsum(np.bincount(cluster, minlength=N_CLUSTERS))]
    )
    wmap = _prep_weights({k: np.asarray(v, dtype=np.float32)
                          for k, v in inputs.items()
                          if k not in ("relative_points", "features", "cluster")})

    in_maps = []
    for k in range(N_CORES):
        m = _prep_core(k, plan, rel_points, features, sort_idx, bucket0)
        m.update(wmap)
        in_maps.append(m)

    res = run_bass_kernel_spmd(nc, in_maps, list(range(N_CORES)),
                               trace=trace, **spmd_kwargs)

    out = np.empty((N_CLUSTERS, 256), dtype=np.float32)
    for k in range(N_CORES):
        out[plan["cids"][k]] = res.results[k]["out"].T
    return out, res


def kernel(**inputs):
    return _run(inputs)[0]
